# revision 15
# baseline (speedup 1.0000x reference)
"""Trainium2 kernel for nn_MemoryMolecular retrieval_knn.

reference:
    logits = x @ feature_queue.T          # [2048, 65536] fp32
    pos = rep_queue[argmax(logits, -1)]; neg = rep_queue[argmin(logits, -1)]

Strategy: shard K across the 8 NeuronCores (8192 columns each).  The host
quantizes x / feature_queue to fp8e4m3; each core computes its logit shard
with DoubleRow fp8 matmuls and streams the logits back as fp8; the host then
exactly rescores (fp64) every candidate within a margin that covers the total
quantization error, recovering the exact fp32 argmax/argmin before gathering
rep_queue rows.

Device schedule (per core, at the fp8-DoubleRow PE roofline of ~110us):
  - x is the stationary operand ([128, 2, 128] DoubleRow blocks); for each of
    16 row tiles x 2 half-groups, ONE weight load serves 8 consecutive N=512
    matmuls into 8 single-bank PSUM tiles (LDWEIGHTS amortized 8x -- the
    previous schedule alternated weights every matmul and was LDW-bound).
  - Two DoubleRow contraction passes (j=0,2) accumulate the full F=512 dot.
  - PSUM->SBUF fp32->fp8 casts split ~50/50 between the Scalar and Vector
    engines; each 4096-col half-group is DMA'd to HBM as soon as it is cast
    (queues alternate sync/gpsimd), hiding the 16.7MB/core output under the
    matmul stream (lbufs=5 keeps enough tiles in flight).
  - The benchmark repeat loop is unrolled up to 4x inside tc.For_i so the
    all-engine loop barrier (~2.5us) amortizes and iterations pipeline.
"""
import numpy as np
import concourse.bacc as bacc
import concourse.mybir as mybir
import concourse.tile as tile
from concourse.bass_utils import run_bass_kernel_spmd

B, K, F = 2048, 65536, 512
NCORES = 8
KS = K // NCORES          # 8192 columns per core
NF = F // 128             # 4 contraction blocks of 128
NT = B // 128             # 16 row tiles
E4 = mybir.dt.float8e4
PACK_QW = 1024            # fq packing group width (must match build_nc qw)
MARGIN = 32.0             # host rescore margin, covers fp8 in+out quantization
                          # (measured max |fp8 path - exact| = 8.5 on 16.8M
                          # sampled entries; 32 gives ~1.9x headroom on the
                          # worst-case pairwise bound)

_nc_cache = None


def build_nc(nt=NT, repeat=1, mode="full", qw=PACK_QW, ppbufs=8, lbufs=5, act_frac=0.5, odt=E4, skip_dma=False, MW=512, dma_split=0, dma_engs=("sync", "gpsimd"), nmm=None):
    """v2 schedule: per (row-tile t, half h of 4096 cols) hold 8 single-bank
    PSUM tiles [128,512]; loop j (DoubleRow contraction pair) OUTSIDE the
    8 column chunks so each stationary weight serves 8 consecutive N=512
    matmuls (LDWEIGHTS amortized 8x).  Evictions (fp32->fp8 cast) split
    between Scalar and Vector engines; per-tile DMA alternates queues."""
    nc = bacc.Bacc("TRN2")
    xtd = nc.dram_tensor("xt", [128, NF * B], E4, kind="ExternalInput")
    fqd = nc.dram_tensor("fq", [128, NF * KS], E4, kind="ExternalInput")
    lbd = nc.dram_tensor("lb", [128, NT * KS], odt, kind="ExternalOutput")

    NMM = nmm if nmm is not None else ppbufs  # matmuls per weight load
    HW_ = NMM * MW            # columns per half-group
    nh = KS // HW_            # half-groups per row tile
    with tile.TileContext(nc) as tc:
        with (
            tc.tile_pool(name="fqp", bufs=1) as fqp,
            tc.tile_pool(name="xp", bufs=1) as xp,
            tc.tile_pool(name="pp", bufs=ppbufs, space="PSUM") as pp,
            tc.tile_pool(name="lp", bufs=lbufs) as lp,
        ):
            ng = KS // qw
            fq = fqp.tile([128, NF * KS], E4)
            xt = xp.tile([128, NF * B], E4)
            nc.sync.dma_start(out=xt[:], in_=xtd[:])
            gw = NF * qw
            for g in range(ng):
                nc.sync.dma_start(out=fq[:, g * gw:(g + 1) * gw],
                                  in_=fqd[:, g * gw:(g + 1) * gw])
            fq4 = fq[:].rearrange("p (g f k) -> p g f k", g=ng, f=NF)
            xt3 = xt[:].rearrange("p (f b) -> p f b", f=NF)

            unroll = 1
            if repeat > 1:
                for u in (4, 3, 2):
                    if repeat % u == 0:
                        unroll = u
                        break
                loop_ctx = tc.For_i(0, repeat // unroll, 1)
                loop_ctx.__enter__()
            s_ = 0
            for tt in range(nt * (unroll if repeat > 1 else 1)):
                t = tt % nt
                Lt = lp.tile([128, KS], odt, name=f"Lt{tt}", tag="Lt")
                for h in range(nh):
                    pts = [pp.tile([128, MW], mybir.dt.float32,
                                   name=f"pt{tt}_{h}_{i}", tag="pt")
                           for i in range(NMM)]
                    for j in range(0, NF, 2):
                        for i in range(NMM):
                            col = h * HW_ + i * MW
                            g, ko = divmod(col, qw)
                            nc.tensor.matmul(
                                pts[i][:],
                                xt3[:, j:j + 2, t * 128:(t + 1) * 128],
                                fq4[:, g, j:j + 2, ko:ko + MW],
                                start=(j == 0), stop=(j == NF - 2),
                                perf_mode=mybir.MatmulPerfMode.DoubleRow,
                            )
                    for i in range(NMM):
                        if mode == "mm":
                            s_ += 1
                            continue
                        col = h * HW_ + i * MW
                        Lb = Lt[:, col:col + MW]
                        if int((s_ + 1) * act_frac) > int(s_ * act_frac):
                            nc.scalar.copy(Lb, pts[i][:])
                        else:
                            nc.vector.tensor_copy(Lb, pts[i][:])
                        s_ += 1
                    if dma_split == 0 and not skip_dma:
                        # per-half DMA: drain each 4096-col half right away
                        dmaeng = getattr(nc, dma_engs[(tt * nh + h) % len(dma_engs)])
                        dmaeng.dma_start(out=lbd[:, t * KS + h * HW_:t * KS + (h + 1) * HW_],
                                         in_=Lt[:, h * HW_:(h + 1) * HW_])
                if dma_split > 0 and (not skip_dma or t == nt - 1):
                    DW = KS // dma_split
                    for d in range(dma_split):
                        dmaeng = getattr(nc, dma_engs[(t * dma_split + d) % len(dma_engs)])
                        dmaeng.dma_start(out=lbd[:, t * KS + d * DW:t * KS + (d + 1) * DW],
                                         in_=Lt[:, d * DW:(d + 1) * DW])
            if repeat > 1:
                loop_ctx.__exit__(None, None, None)
    nc.compile()
    return nc


def _pack_inputs(x, feature_queue):
    """fp8-quantize and pack [*, F] operands as [128, NF * n] f-blocked."""
    e4 = mybir.dt.np(E4)
    xT = np.ascontiguousarray(
        x.T.reshape(NF, 128, B).transpose(1, 0, 2).reshape(128, NF * B)).astype(e4)
    fq_packs = []
    G = KS // PACK_QW
    for c in range(NCORES):
        shard = feature_queue[c * KS:(c + 1) * KS]      # [KS, F]
        fqT = np.ascontiguousarray(
            shard.T.reshape(NF, 128, G, PACK_QW).transpose(1, 2, 0, 3)
            .reshape(128, NF * KS)).astype(e4)
        fq_packs.append(fqT)
    return xT, fq_packs


def _assemble_logits(results):
    """[core][128, NT*KS] fp8 -> [B, K] float32 (row b = t*128+p)."""
    cols = []
    for r in results:
        lb = np.asarray(r["lb"])                       # [128, NT*KS] fp8
        lb = lb.reshape(128, NT, KS).transpose(1, 0, 2).reshape(B, KS)
        cols.append(lb.astype(np.float32))
    return np.concatenate(cols, axis=1)                # [B, K] f32


def _exact_pick(x, feature_queue, approx, mode):
    """Exact argmax/argmin: rescore all candidates within MARGIN of the
    approx extreme with an fp64 dot; ties -> smallest index."""
    if mode == "max":
        ext = approx.max(axis=1, keepdims=True)
        rows, cands = np.nonzero(approx >= ext - MARGIN)
    else:
        ext = approx.min(axis=1, keepdims=True)
        rows, cands = np.nonzero(approx <= ext + MARGIN)
    scores = np.einsum("if,if->i", x[rows].astype(np.float64),
                       feature_queue[cands].astype(np.float64))
    out = np.empty(B, dtype=np.int64)
    starts = np.searchsorted(rows, np.arange(B))
    ends = np.searchsorted(rows, np.arange(B), side="right")
    for b in range(B):
        s, e = starts[b], ends[b]
        sc = scores[s:e]
        ks = cands[s:e]
        top = sc.max() if mode == "max" else sc.min()
        out[b] = ks[sc == top].min()
    return out


def kernel(x, feature_queue, rep_queue):
    global _nc_cache
    x = np.asarray(x, dtype=np.float32)
    feature_queue = np.asarray(feature_queue, dtype=np.float32)
    rep_queue = np.asarray(rep_queue, dtype=np.float32)

    if _nc_cache is None:
        _nc_cache = build_nc()
    nc = _nc_cache

    xT, fq_packs = _pack_inputs(x, feature_queue)
    in_maps = [{"xt": xT, "fq": fq_packs[c]} for c in range(NCORES)]
    results = run_bass_kernel_spmd(nc, in_maps, core_ids=list(range(NCORES))).results

    approx = _assemble_logits(results)
    pos_idx = _exact_pick(x, feature_queue, approx, "max")
    neg_idx = _exact_pick(x, feature_queue, approx, "min")
    return (rep_queue[pos_idx], rep_queue[neg_idx])



# revision 16
# speedup vs baseline: 1.0669x; 1.0669x over previous
"""Trainium2 kernel for nn_MemoryMolecular retrieval_knn.

reference:
    logits = x @ feature_queue.T          # [2048, 65536] fp32
    pos = rep_queue[argmax(logits, -1)]; neg = rep_queue[argmin(logits, -1)]

Strategy: shard K across the 8 NeuronCores (8192 columns each).  The host
quantizes x / feature_queue to fp8e4m3; each core computes its logit shard
with DoubleRow fp8 matmuls and streams the logits back as fp8; the host then
exactly rescores (fp64) every candidate within a margin that covers the total
quantization error, recovering the exact fp32 argmax/argmin before gathering
rep_queue rows.

Device schedule (per core, at the fp8-DoubleRow PE roofline of ~110us):
  - x is the stationary operand ([128, 2, 128] DoubleRow blocks); for each of
    16 row tiles x 2 half-groups, ONE weight load serves 8 consecutive N=512
    matmuls into 8 single-bank PSUM tiles (LDWEIGHTS amortized 8x -- the
    previous schedule alternated weights every matmul and was LDW-bound).
  - Two DoubleRow contraction passes (j=0,2) accumulate the full F=512 dot.
  - PSUM->SBUF fp32->fp8 casts split ~50/50 between the Scalar and Vector
    engines; each 4096-col half-group is DMA'd to HBM as soon as it is cast
    (queues alternate sync/gpsimd), hiding the 16.7MB/core output under the
    matmul stream (lbufs=5 keeps enough tiles in flight).
  - The benchmark repeat loop is unrolled up to 4x inside tc.For_i so the
    all-engine loop barrier (~2.5us) amortizes and iterations pipeline.
"""
import numpy as np
import concourse.bacc as bacc
import concourse.mybir as mybir
import concourse.tile as tile
from concourse.bass_utils import run_bass_kernel_spmd

B, K, F = 2048, 65536, 512
NCORES = 8
KS = K // NCORES          # 8192 columns per core
NF = F // 128             # 4 contraction blocks of 128
NT = B // 128             # 16 row tiles
E4 = mybir.dt.float8e4
PACK_QW = 1024            # fq packing group width (must match build_nc qw)
MARGIN = 32.0             # host rescore margin, covers fp8 in+out quantization
                          # (measured max |fp8 path - exact| = 8.5 on 16.8M
                          # sampled entries; 32 gives ~1.9x headroom on the
                          # worst-case pairwise bound)

_nc_cache = None


def build_nc(nt=NT, repeat=1, mode="full", qw=PACK_QW, ppbufs=8, lbufs=5, act_frac=0.5, odt=E4, skip_dma=False, MW=512, dma_split=0, dma_engs=("sync", "gpsimd"), nmm=None):
    """v2 schedule: per (row-tile t, half h of 4096 cols) hold 8 single-bank
    PSUM tiles [128,512]; loop j (DoubleRow contraction pair) OUTSIDE the
    8 column chunks so each stationary weight serves 8 consecutive N=512
    matmuls (LDWEIGHTS amortized 8x).  Evictions (fp32->fp8 cast) split
    between Scalar and Vector engines; per-tile DMA alternates queues."""
    nc = bacc.Bacc("TRN2")
    xtd = nc.dram_tensor("xt", [128, NF * B], E4, kind="ExternalInput")
    fqd = nc.dram_tensor("fq", [128, NF * KS], E4, kind="ExternalInput")
    lbd = nc.dram_tensor("lb", [128, NT * KS], odt, kind="ExternalOutput")

    NMM = nmm if nmm is not None else ppbufs  # matmuls per weight load
    HW_ = NMM * MW            # columns per half-group
    nh = KS // HW_            # half-groups per row tile
    with tile.TileContext(nc) as tc:
        with (
            tc.tile_pool(name="fqp", bufs=1) as fqp,
            tc.tile_pool(name="xp", bufs=1) as xp,
            tc.tile_pool(name="pp", bufs=ppbufs, space="PSUM") as pp,
            tc.tile_pool(name="lp", bufs=lbufs) as lp,
        ):
            ng = KS // qw
            fq = fqp.tile([128, NF * KS], E4)
            xt = xp.tile([128, NF * B], E4)
            nc.sync.dma_start(out=xt[:], in_=xtd[:])
            gw = NF * qw
            for g in range(ng):
                nc.sync.dma_start(out=fq[:, g * gw:(g + 1) * gw],
                                  in_=fqd[:, g * gw:(g + 1) * gw])
            fq4 = fq[:].rearrange("p (g f k) -> p g f k", g=ng, f=NF)
            xt3 = xt[:].rearrange("p (f b) -> p f b", f=NF)

            unroll = 1
            if repeat > 1:
                for u in (8, 6, 4, 3, 2):
                    if repeat % u == 0:
                        unroll = u
                        break
                loop_ctx = tc.For_i(0, repeat // unroll, 1)
                loop_ctx.__enter__()
            s_ = 0
            for tt in range(nt * (unroll if repeat > 1 else 1)):
                t = tt % nt
                Lt = lp.tile([128, KS], odt, name=f"Lt{tt}", tag="Lt")
                for h in range(nh):
                    pts = [pp.tile([128, MW], mybir.dt.float32,
                                   name=f"pt{tt}_{h}_{i}", tag="pt")
                           for i in range(NMM)]
                    for j in range(0, NF, 2):
                        for i in range(NMM):
                            col = h * HW_ + i * MW
                            g, ko = divmod(col, qw)
                            nc.tensor.matmul(
                                pts[i][:],
                                xt3[:, j:j + 2, t * 128:(t + 1) * 128],
                                fq4[:, g, j:j + 2, ko:ko + MW],
                                start=(j == 0), stop=(j == NF - 2),
                                perf_mode=mybir.MatmulPerfMode.DoubleRow,
                            )
                    for i in range(NMM):
                        if mode == "mm":
                            s_ += 1
                            continue
                        col = h * HW_ + i * MW
                        Lb = Lt[:, col:col + MW]
                        if int((s_ + 1) * act_frac) > int(s_ * act_frac):
                            nc.scalar.copy(Lb, pts[i][:])
                        else:
                            nc.vector.tensor_copy(Lb, pts[i][:])
                        s_ += 1
                    if dma_split == 0 and not skip_dma:
                        # per-half DMA: drain each 4096-col half right away
                        dmaeng = getattr(nc, dma_engs[(tt * nh + h) % len(dma_engs)])
                        dmaeng.dma_start(out=lbd[:, t * KS + h * HW_:t * KS + (h + 1) * HW_],
                                         in_=Lt[:, h * HW_:(h + 1) * HW_])
                if dma_split > 0 and (not skip_dma or t == nt - 1):
                    DW = KS // dma_split
                    for d in range(dma_split):
                        dmaeng = getattr(nc, dma_engs[(t * dma_split + d) % len(dma_engs)])
                        dmaeng.dma_start(out=lbd[:, t * KS + d * DW:t * KS + (d + 1) * DW],
                                         in_=Lt[:, d * DW:(d + 1) * DW])
            if repeat > 1:
                loop_ctx.__exit__(None, None, None)
    nc.compile()
    return nc


def _pack_inputs(x, feature_queue):
    """fp8-quantize and pack [*, F] operands as [128, NF * n] f-blocked."""
    e4 = mybir.dt.np(E4)
    xT = np.ascontiguousarray(
        x.T.reshape(NF, 128, B).transpose(1, 0, 2).reshape(128, NF * B)).astype(e4)
    fq_packs = []
    G = KS // PACK_QW
    for c in range(NCORES):
        shard = feature_queue[c * KS:(c + 1) * KS]      # [KS, F]
        fqT = np.ascontiguousarray(
            shard.T.reshape(NF, 128, G, PACK_QW).transpose(1, 2, 0, 3)
            .reshape(128, NF * KS)).astype(e4)
        fq_packs.append(fqT)
    return xT, fq_packs


def _assemble_logits(results):
    """[core][128, NT*KS] fp8 -> [B, K] float32 (row b = t*128+p)."""
    cols = []
    for r in results:
        lb = np.asarray(r["lb"])                       # [128, NT*KS] fp8
        lb = lb.reshape(128, NT, KS).transpose(1, 0, 2).reshape(B, KS)
        cols.append(lb.astype(np.float32))
    return np.concatenate(cols, axis=1)                # [B, K] f32


def _exact_pick(x, feature_queue, approx, mode):
    """Exact argmax/argmin: rescore all candidates within MARGIN of the
    approx extreme with an fp64 dot; ties -> smallest index."""
    if mode == "max":
        ext = approx.max(axis=1, keepdims=True)
        rows, cands = np.nonzero(approx >= ext - MARGIN)
    else:
        ext = approx.min(axis=1, keepdims=True)
        rows, cands = np.nonzero(approx <= ext + MARGIN)
    scores = np.einsum("if,if->i", x[rows].astype(np.float64),
                       feature_queue[cands].astype(np.float64))
    out = np.empty(B, dtype=np.int64)
    starts = np.searchsorted(rows, np.arange(B))
    ends = np.searchsorted(rows, np.arange(B), side="right")
    for b in range(B):
        s, e = starts[b], ends[b]
        sc = scores[s:e]
        ks = cands[s:e]
        top = sc.max() if mode == "max" else sc.min()
        out[b] = ks[sc == top].min()
    return out


def kernel(x, feature_queue, rep_queue):
    global _nc_cache
    x = np.asarray(x, dtype=np.float32)
    feature_queue = np.asarray(feature_queue, dtype=np.float32)
    rep_queue = np.asarray(rep_queue, dtype=np.float32)

    if _nc_cache is None:
        _nc_cache = build_nc()
    nc = _nc_cache

    xT, fq_packs = _pack_inputs(x, feature_queue)
    in_maps = [{"xt": xT, "fq": fq_packs[c]} for c in range(NCORES)]
    results = run_bass_kernel_spmd(nc, in_maps, core_ids=list(range(NCORES))).results

    approx = _assemble_logits(results)
    pos_idx = _exact_pick(x, feature_queue, approx, "max")
    neg_idx = _exact_pick(x, feature_queue, approx, "min")
    return (rep_queue[pos_idx], rep_queue[neg_idx])

